# revision 25
# baseline (speedup 1.0000x reference)
"""Trainium2 Bass kernel for nn_DeltaEncoderBlock.

Reference semantics (all fp32):
    x: [64, 9, 14, 384] -> x_flat [64, 126, 384]
    delta[t] = x[t] - x[t-1]  (delta[0] = x[0])        (temporal delta)
    w = g * v / ||v||_row                               (weight norm, [1024, 126])
    z = einsum('oi,bit->tbo', w, delta)                 (synaptic input)
    scan over t:  cur = 0.75*cur + z_t
                  vol = 0.97*vol + cur
                  s   = (vol >= 1)
                  vol = vol * (1 - s)                   (hard reset)
    out: spikes [64, 1024, 384]

Sharding: data-parallel over batch across 8 NeuronCores (8 batches/core).

Per-core kernel:
  - z via PE fp32 matmuls (K=126), o in 8 chunks of 128 partitions,
    weight-norm scale applied in the PSUM->SBUF copy on ScalarE.
  - cur via DVE tensor_tensor_scan (linear recurrence along t).
  - vol/spike loop: 2 fused scalar_tensor_tensor DVE ops per step;
    spike = Relu(Sign(vol_pre - 1)) on ScalarE (Sign per step, Relu per
    48-step block), DMA'd out per block.
"""

import numpy as np

import concourse.bacc as bacc
import concourse.tile as tile
from concourse import mybir
from concourse.bass_utils import run_bass_kernel_spmd
from concourse.masks import make_identity

N_CORES = 8
B, C, H, T = 64, 9, 14, 384
I = C * H  # 126
O = 1024
BL = B // N_CORES  # 8 batches per core
NCH = O // 128  # 8 o-chunks of 128
TBLK = 64  # t-block: matmul window, z tile span, and spike staging block
NTB = T // TBLK  # 6
F32 = mybir.dt.float32

CURRENT_DECAY = 0.25
VOLTAGE_DECAY = 0.03

# fp32r streams fp32 through the PE at bf16 rate (4x faster than plain fp32
# matmul); numerics differ slightly from fp32 — gated on a HW accuracy check.
MM_F32R = False


def _body(tc, x, v, g, out):
    nc = tc.nc
    Alu = mybir.AluOpType
    Act = mybir.ActivationFunctionType

    import contextlib

    with contextlib.ExitStack() as ctx:
        consts = ctx.enter_context(tc.tile_pool(name="consts", bufs=1))
        big = ctx.enter_context(tc.tile_pool(name="big", bufs=1))
        wp = ctx.enter_context(tc.tile_pool(name="wp", bufs=1))
        psT = ctx.enter_context(tc.tile_pool(name="psT", bufs=2, space="PSUM"))
        psZ = ctx.enter_context(tc.tile_pool(name="psZ", bufs=6, space="PSUM"))
        pvolS = ctx.enter_context(tc.tile_pool(name="pvolS", bufs=8))
        pstage = ctx.enter_context(tc.tile_pool(name="pstage", bufs=2))

        # ---- load x per batch, pipelined with delta + cur-delta scan ----
        # cur-delta: scan the 0.75 recurrence on delta before the matmul;
        # W.(scan delta) == scan (W.delta) by linearity.
        decD = consts.tile([I, 1], F32)
        nc.vector.memset(decD[:], 1.0 - CURRENT_DECAY)
        xs = big.tile([I, BL * T], F32)
        x3 = xs[:].rearrange("p (b t) -> p b t", b=BL)
        delta = big.tile([I, BL * T], F32)
        d3 = delta[:].rearrange("p (b t) -> p b t", b=BL)
        TH = 2 * TBLK  # scan in halves: first half unblocks early matmuls
        for b in range(BL):
            nc.sync.dma_start(x3[:, b, :], x[b].rearrange("i t -> i t"))
            nc.vector.tensor_copy(d3[:, b, 0:1], x3[:, b, 0:1])
            nc.vector.tensor_tensor(
                out=d3[:, b, 1:TH], in0=x3[:, b, 1:TH], in1=x3[:, b, 0 : TH - 1],
                op=Alu.subtract,
            )
            seg = delta[:, b * T : b * T + TH]
            nc.vector.tensor_tensor_scan(
                seg, decD[:].to_broadcast([I, TH]), seg, 0.0, Alu.mult, Alu.add
            )
        for b in range(BL):
            nc.vector.tensor_tensor(
                out=d3[:, b, TH:T], in0=x3[:, b, TH:T], in1=x3[:, b, TH - 1 : T - 1],
                op=Alu.subtract,
            )
            seg = delta[:, b * T + TH : (b + 1) * T]
            carry = delta[:, b * T + TH - 1 : b * T + TH]
            nc.vector.tensor_tensor_scan(
                seg, decD[:].to_broadcast([I, T - TH]), seg, carry,
                Alu.mult, Alu.add,
            )

        # ---- weights: v natural layout, norms, transpose ----
        vt = wp.tile([128, NCH * I], F32)
        vt3 = vt[:].rearrange("p (c i) -> p c i", c=NCH)
        nc.sync.dma_start(vt3, v.rearrange("(c p) i -> p c i", p=128))

        gt = wp.tile([128, NCH], F32)
        nc.sync.dma_start(gt[:], g.rearrange("(c p) -> p c", p=128))

        sq = wp.tile([128, I], F32)  # scratch for Square
        ss = wp.tile([128, NCH], F32)  # row sum-of-squares
        for c in range(NCH):
            nc.scalar.activation(
                sq[:], vt3[:, c, :], Act.Square, accum_out=ss[:, c : c + 1]
            )
        inv = wp.tile([128, NCH], F32)
        nc.vector.reciprocal(inv[:], ss[:])
        rs = wp.tile([128, NCH], F32)
        nc.scalar.sqrt(rs[:], inv[:])  # rsqrt(sum v^2)
        scale = wp.tile([128, NCH], F32)
        nc.vector.tensor_tensor(out=scale[:], in0=rs[:], in1=gt[:], op=Alu.mult)

        ident = consts.tile([128, 128], F32)
        make_identity(nc, ident[:])

        wT = []  # per-chunk [126, 128] tiles of v^T
        for c in range(NCH):
            pt = psT.tile([I, 128], F32)
            nc.tensor.transpose(pt[:], vt3[:, c, :], ident[:])
            wc = wp.tile([I, 128], F32, tag=f"wT{c}")
            nc.scalar.copy(wc[:], pt[:])
            wT.append(wc)

        # ---- cur = (v^T . cur-delta), scaled by g/||v|| on the PSUM->SBUF
        # copy. One z tile per t-block of TBLK steps, layout [p, (c b tl)],
        # so the vol loop starts after the first t-block's matmuls and the
        # rest of the matmul phase hides under the loop. Matmul windows
        # enumerate (tl, b) columns via a strided rhs AP on delta. ----
        dly = delta[:].rearrange("p (b t) -> p t b", b=BL)  # [126, T, BL]
        ztiles = []
        for tb in range(NTB):
            zt = big.tile([128, NCH * BL * TBLK], F32, tag=f"z{tb}")
            ztiles.append(zt)
            # memory layout (c, b, tl); dims permuted to enumerate (tl, b)
            zv = zt[:].rearrange("p (c b tl) -> p c tl b", c=NCH, b=BL)
            for c in range(NCH):
                ps = psZ.tile([128, BL * TBLK], F32)
                mm_lhs = wT[c][:]
                mm_rhs = dly[:, tb * TBLK : (tb + 1) * TBLK, :]
                if MM_F32R:
                    mm_lhs = mm_lhs.bitcast(mybir.dt.float32r)
                    mm_rhs = mm_rhs.bitcast(mybir.dt.float32r)
                nc.tensor.matmul(
                    ps[:], lhsT=mm_lhs, rhs=mm_rhs, start=True, stop=True
                )
                # psum cols are (tl, b); write them to z at (b*TBLK + tl)
                nc.scalar.activation(
                    zv[:, c, :, :],
                    ps[:].rearrange("p (tl b) -> p tl b", b=BL),
                    Act.Copy,
                    scale=scale[:, c : c + 1],
                )

        # ---- vol loop: vol_pre overwrites the cur column of z in place.
        # DVE-only; no cross-engine sync inside the loop. ----
        neg1 = consts.tile([128, 1], F32)
        nc.vector.memset(neg1[:], -1.0)
        volS = pvolS.tile([128, NCH * BL], F32, tag="volS")
        nc.vector.memset(volS[:], 0.0)
        vdec = 1.0 - VOLTAGE_DECAY

        out_r = out.rearrange("b (c p) t -> p c b t", c=NCH)
        for t in range(T):
            tb, tl = divmod(t, TBLK)
            zc = ztiles[tb][:].rearrange(
                "p (c b tl) -> p c b tl", c=NCH, b=BL
            )
            # vol_pre = vdec * vol + cur_t   (written over cur_t)
            nc.vector.scalar_tensor_tensor(
                zc[:, :, :, tl],
                volS[:].rearrange("p (c b) -> p c b", c=NCH),
                vdec,
                zc[:, :, :, tl],
                Alu.mult,
                Alu.add,
            )
            # vol = (vol_pre < 1) * vol_pre   (hard reset)
            volS = pvolS.tile([128, NCH * BL], F32, tag="volS")
            nc.vector.scalar_tensor_tensor(
                volS[:].rearrange("p (c b) -> p c b", c=NCH),
                zc[:, :, :, tl],
                1.0,
                zc[:, :, :, tl],
                Alu.is_lt,
                Alu.mult,
            )
            # spikes on ACT (off the DVE path): Sign -> in-place Relu ->
            # DMA out, staging (c, b, tl). Block tb-1 is extracted at the
            # START of block tb (its data is complete and the ACT ops are
            # immediately ready); the final block flushes in quarters as
            # its columns finish so only the last quarter sits on the tail.
            flush = []
            if tb >= 1 and tl == 0:
                flush = [(tb - 1, 0, TBLK)]
            if tb == NTB - 1 and (tl + 1) % (TBLK // 4) == 0:
                q = (tl + 1) // (TBLK // 4) - 1
                flush += [(tb, q * (TBLK // 4), (q + 1) * (TBLK // 4))]
            for ftb, lo, hi in flush:
                w = hi - lo
                zcf = ztiles[ftb][:].rearrange(
                    "p (c b tl) -> p c b tl", c=NCH, b=BL
                )
                sstage = pstage.tile([128, 64 * w], F32, tag=f"ss{w}")
                s3 = sstage[:].rearrange(
                    "p (c b tl) -> p c b tl", c=NCH, b=BL
                )
                nc.scalar.activation(
                    s3, zcf[:, :, :, lo:hi], Act.Sign, bias=neg1[:]
                )
                nc.scalar.activation(sstage[:], sstage[:], Act.Relu)
                for c in range(NCH):
                    nc.sync.dma_start(
                        out_r[:, c, :, ftb * TBLK + lo : ftb * TBLK + hi],
                        s3[:, c, :, :],
                    )


_CACHE = {}


def _build():
    if "nc" in _CACHE:
        return _CACHE["nc"]
    nc = bacc.Bacc(
        "TRN2", target_bir_lowering=False, debug=False, num_devices=N_CORES
    )
    x = nc.dram_tensor("x", [BL, I, T], F32, kind="ExternalInput").ap()
    v = nc.dram_tensor("v", [O, I], F32, kind="ExternalInput").ap()
    g = nc.dram_tensor("g", [O], F32, kind="ExternalInput").ap()
    out = nc.dram_tensor("out", [BL, O, T], F32, kind="ExternalOutput").ap()
    with tile.TileContext(nc) as tc:
        _body(tc, x, v, g, out)
    nc.compile()
    _CACHE["nc"] = nc
    return nc


def make_in_maps(x, v_weight, g):
    xr = np.ascontiguousarray(x.reshape(B, I, T))
    v_weight = np.ascontiguousarray(v_weight)
    g = np.ascontiguousarray(g)
    return [
        {
            "x": np.ascontiguousarray(xr[c * BL : (c + 1) * BL]),
            "v": v_weight,
            "g": g,
        }
        for c in range(N_CORES)
    ]


def kernel(x, v_weight, g):
    nc = _build()
    in_maps = make_in_maps(
        np.asarray(x, dtype=np.float32),
        np.asarray(v_weight, dtype=np.float32),
        np.asarray(g, dtype=np.float32),
    )
    last_err = None
    for _attempt in range(3):  # retry: a prior tenant can leave a core wedged
        try:
            res = run_bass_kernel_spmd(nc, in_maps, list(range(N_CORES))).results
            return np.concatenate(
                [res[c]["out"] for c in range(N_CORES)], axis=0
            )
        except Exception as e:  # noqa: BLE001
            last_err = e
    raise last_err


# revision 29
# speedup vs baseline: 1.0092x; 1.0092x over previous
"""Trainium2 Bass kernel for nn_DeltaEncoderBlock.

Reference semantics (all fp32):
    x: [64, 9, 14, 384] -> x_flat [64, 126, 384]
    delta[t] = x[t] - x[t-1]  (delta[0] = x[0])        (temporal delta)
    w = g * v / ||v||_row                               (weight norm, [1024, 126])
    z = einsum('oi,bit->tbo', w, delta)                 (synaptic input)
    scan over t:  cur = 0.75*cur + z_t
                  vol = 0.97*vol + cur
                  s   = (vol >= 1)
                  vol = vol * (1 - s)                   (hard reset)
    out: spikes [64, 1024, 384]

Sharding: data-parallel over batch across 8 NeuronCores (8 batches/core).

Per-core kernel:
  - z via PE fp32 matmuls (K=126), o in 8 chunks of 128 partitions,
    weight-norm scale applied in the PSUM->SBUF copy on ScalarE.
  - cur via DVE tensor_tensor_scan (linear recurrence along t).
  - vol/spike loop: 2 fused scalar_tensor_tensor DVE ops per step;
    spike = Relu(Sign(vol_pre - 1)) on ScalarE (Sign per step, Relu per
    48-step block), DMA'd out per block.
"""

import numpy as np

import concourse.bacc as bacc
import concourse.tile as tile
from concourse import mybir
from concourse.bass_utils import run_bass_kernel_spmd
from concourse.masks import make_identity

N_CORES = 8
B, C, H, T = 64, 9, 14, 384
I = C * H  # 126
O = 1024
BL = B // N_CORES  # 8 batches per core
NCH = O // 128  # 8 o-chunks of 128
TBLK = 64  # t-block: matmul window, z tile span, and spike staging block
NTB = T // TBLK  # 6
F32 = mybir.dt.float32
U8 = mybir.dt.uint8

CURRENT_DECAY = 0.25
VOLTAGE_DECAY = 0.03

# fp32r streams fp32 through the PE at bf16 rate (4x faster than plain fp32
# matmul); numerics differ slightly from fp32 — gated on a HW accuracy check.
MM_F32R = False


def _body(tc, x, v, g, out):
    nc = tc.nc
    Alu = mybir.AluOpType
    Act = mybir.ActivationFunctionType

    import contextlib

    with contextlib.ExitStack() as ctx:
        consts = ctx.enter_context(tc.tile_pool(name="consts", bufs=1))
        big = ctx.enter_context(tc.tile_pool(name="big", bufs=1))
        wp = ctx.enter_context(tc.tile_pool(name="wp", bufs=1))
        psT = ctx.enter_context(tc.tile_pool(name="psT", bufs=2, space="PSUM"))
        psZ = ctx.enter_context(tc.tile_pool(name="psZ", bufs=6, space="PSUM"))
        pvolS = ctx.enter_context(tc.tile_pool(name="pvolS", bufs=8))
        pstage = ctx.enter_context(tc.tile_pool(name="pstage", bufs=2))

        # ---- load x per batch, pipelined with delta + cur-delta scan ----
        # cur-delta: scan the 0.75 recurrence on delta before the matmul;
        # W.(scan delta) == scan (W.delta) by linearity.
        decD = consts.tile([I, 1], F32)
        nc.vector.memset(decD[:], 1.0 - CURRENT_DECAY)
        xs = big.tile([I, BL * T], F32)
        x3 = xs[:].rearrange("p (b t) -> p b t", b=BL)
        delta = big.tile([I, BL * T], F32)
        d3 = delta[:].rearrange("p (b t) -> p b t", b=BL)
        TH = 2 * TBLK  # scan in halves: first half unblocks early matmuls
        for b in range(BL):
            nc.sync.dma_start(x3[:, b, :], x[b].rearrange("i t -> i t"))
            nc.vector.tensor_copy(d3[:, b, 0:1], x3[:, b, 0:1])
            nc.vector.tensor_tensor(
                out=d3[:, b, 1:TH], in0=x3[:, b, 1:TH], in1=x3[:, b, 0 : TH - 1],
                op=Alu.subtract,
            )
            seg = delta[:, b * T : b * T + TH]
            nc.vector.tensor_tensor_scan(
                seg, decD[:].to_broadcast([I, TH]), seg, 0.0, Alu.mult, Alu.add
            )
        for b in range(BL):
            nc.vector.tensor_tensor(
                out=d3[:, b, TH:T], in0=x3[:, b, TH:T], in1=x3[:, b, TH - 1 : T - 1],
                op=Alu.subtract,
            )
            seg = delta[:, b * T + TH : (b + 1) * T]
            carry = delta[:, b * T + TH - 1 : b * T + TH]
            nc.vector.tensor_tensor_scan(
                seg, decD[:].to_broadcast([I, T - TH]), seg, carry,
                Alu.mult, Alu.add,
            )

        # ---- weights: v natural layout, norms, transpose ----
        vt = wp.tile([128, NCH * I], F32)
        vt3 = vt[:].rearrange("p (c i) -> p c i", c=NCH)
        nc.sync.dma_start(vt3, v.rearrange("(c p) i -> p c i", p=128))

        gt = wp.tile([128, NCH], F32)
        nc.sync.dma_start(gt[:], g.rearrange("(c p) -> p c", p=128))

        sq = wp.tile([128, I], F32)  # scratch for Square
        ss = wp.tile([128, NCH], F32)  # row sum-of-squares
        for c in range(NCH):
            nc.scalar.activation(
                sq[:], vt3[:, c, :], Act.Square, accum_out=ss[:, c : c + 1]
            )
        inv = wp.tile([128, NCH], F32)
        nc.vector.reciprocal(inv[:], ss[:])
        rs = wp.tile([128, NCH], F32)
        nc.scalar.sqrt(rs[:], inv[:])  # rsqrt(sum v^2)
        scale = wp.tile([128, NCH], F32)
        nc.vector.tensor_tensor(out=scale[:], in0=rs[:], in1=gt[:], op=Alu.mult)

        ident = consts.tile([128, 128], F32)
        make_identity(nc, ident[:])

        wT = []  # per-chunk [126, 128] tiles of v^T
        for c in range(NCH):
            pt = psT.tile([I, 128], F32)
            nc.tensor.transpose(pt[:], vt3[:, c, :], ident[:])
            wc = wp.tile([I, 128], F32, tag=f"wT{c}")
            nc.scalar.copy(wc[:], pt[:])
            wT.append(wc)

        # ---- cur = (v^T . cur-delta), scaled by g/||v|| on the PSUM->SBUF
        # copy. One z tile per t-block of TBLK steps, layout [p, (c b tl)],
        # so the vol loop starts after the first t-block's matmuls and the
        # rest of the matmul phase hides under the loop. Matmul windows
        # enumerate (tl, b) columns via a strided rhs AP on delta. ----
        dly = delta[:].rearrange("p (b t) -> p t b", b=BL)  # [126, T, BL]
        ztiles = []
        for tb in range(NTB):
            zt = big.tile([128, NCH * BL * TBLK], F32, tag=f"z{tb}")
            ztiles.append(zt)
            # memory layout (c, b, tl); dims permuted to enumerate (tl, b)
            zv = zt[:].rearrange("p (c b tl) -> p c tl b", c=NCH, b=BL)
            for c in range(NCH):
                ps = psZ.tile([128, BL * TBLK], F32)
                mm_lhs = wT[c][:]
                mm_rhs = dly[:, tb * TBLK : (tb + 1) * TBLK, :]
                if MM_F32R:
                    mm_lhs = mm_lhs.bitcast(mybir.dt.float32r)
                    mm_rhs = mm_rhs.bitcast(mybir.dt.float32r)
                nc.tensor.matmul(
                    ps[:], lhsT=mm_lhs, rhs=mm_rhs, start=True, stop=True
                )
                # psum cols are (tl, b); write them to z at (b*TBLK + tl)
                nc.scalar.activation(
                    zv[:, c, :, :],
                    ps[:].rearrange("p (tl b) -> p tl b", b=BL),
                    Act.Copy,
                    scale=scale[:, c : c + 1],
                )

        # ---- vol loop: vol_pre overwrites the cur column of z in place.
        # DVE-only; no cross-engine sync inside the loop. ----
        neg1 = consts.tile([128, 1], F32)
        nc.vector.memset(neg1[:], -1.0)
        volS = pvolS.tile([128, NCH * BL], F32, tag="volS")
        nc.vector.memset(volS[:], 0.0)
        vdec = 1.0 - VOLTAGE_DECAY

        out_r = out.rearrange("b (c p) t -> p c b t", c=NCH)
        for t in range(T):
            tb, tl = divmod(t, TBLK)
            zc = ztiles[tb][:].rearrange(
                "p (c b tl) -> p c b tl", c=NCH, b=BL
            )
            # vol_pre = vdec * vol + cur_t   (written over cur_t)
            nc.vector.scalar_tensor_tensor(
                zc[:, :, :, tl],
                volS[:].rearrange("p (c b) -> p c b", c=NCH),
                vdec,
                zc[:, :, :, tl],
                Alu.mult,
                Alu.add,
            )
            # vol = (vol_pre < 1) * vol_pre   (hard reset)
            volS = pvolS.tile([128, NCH * BL], F32, tag="volS")
            nc.vector.scalar_tensor_tensor(
                volS[:].rearrange("p (c b) -> p c b", c=NCH),
                zc[:, :, :, tl],
                1.0,
                zc[:, :, :, tl],
                Alu.is_lt,
                Alu.mult,
            )
            # spikes on ACT (off the DVE path): Sign -> in-place Relu ->
            # DMA out, staging (c, b, tl). Block tb-1 is extracted at the
            # START of block tb (its data is complete and the ACT ops are
            # immediately ready); the final block flushes in quarters as
            # its columns finish so only the last quarter sits on the tail.
            flush = []
            if tb >= 1 and tl == 0:
                flush = [(tb - 1, 0, TBLK)]
            if tb == NTB - 1 and (tl + 1) % (TBLK // 4) == 0:
                q = (tl + 1) // (TBLK // 4) - 1
                flush += [(tb, q * (TBLK // 4), (q + 1) * (TBLK // 4))]
            for ftb, lo, hi in flush:
                w = hi - lo
                zcf = ztiles[ftb][:].rearrange(
                    "p (c b tl) -> p c b tl", c=NCH, b=BL
                )
                sstage = pstage.tile([128, 64 * w], F32, tag=f"ss{w}")
                s3 = sstage[:].rearrange(
                    "p (c b tl) -> p c b tl", c=NCH, b=BL
                )
                nc.scalar.activation(
                    s3, zcf[:, :, :, lo:hi], Act.Sign, bias=neg1[:]
                )
                # Relu converts {-1,0,1} -> {0,1} and narrows to uint8:
                # 4x fewer DMA bytes (spikes are exact 0/1; host widens).
                ostage = pstage.tile([128, 64 * w], U8, tag=f"os{w}")
                o3 = ostage[:].rearrange(
                    "p (c b tl) -> p c b tl", c=NCH, b=BL
                )
                nc.scalar.activation(o3, s3, Act.Relu)
                for c in range(NCH):
                    nc.sync.dma_start(
                        out_r[:, c, :, ftb * TBLK + lo : ftb * TBLK + hi],
                        o3[:, c, :, :],
                    )


_CACHE = {}


def _build():
    if "nc" in _CACHE:
        return _CACHE["nc"]
    nc = bacc.Bacc(
        "TRN2", target_bir_lowering=False, debug=False, num_devices=N_CORES
    )
    x = nc.dram_tensor("x", [BL, I, T], F32, kind="ExternalInput").ap()
    v = nc.dram_tensor("v", [O, I], F32, kind="ExternalInput").ap()
    g = nc.dram_tensor("g", [O], F32, kind="ExternalInput").ap()
    out = nc.dram_tensor("out", [BL, O, T], U8, kind="ExternalOutput").ap()
    with tile.TileContext(nc) as tc:
        _body(tc, x, v, g, out)
    nc.compile()
    _CACHE["nc"] = nc
    return nc


def make_in_maps(x, v_weight, g):
    xr = np.ascontiguousarray(x.reshape(B, I, T))
    v_weight = np.ascontiguousarray(v_weight)
    g = np.ascontiguousarray(g)
    return [
        {
            "x": np.ascontiguousarray(xr[c * BL : (c + 1) * BL]),
            "v": v_weight,
            "g": g,
        }
        for c in range(N_CORES)
    ]


def kernel(x, v_weight, g):
    nc = _build()
    in_maps = make_in_maps(
        np.asarray(x, dtype=np.float32),
        np.asarray(v_weight, dtype=np.float32),
        np.asarray(g, dtype=np.float32),
    )
    last_err = None
    for _attempt in range(3):  # retry: a prior tenant can leave a core wedged
        try:
            res = run_bass_kernel_spmd(nc, in_maps, list(range(N_CORES))).results
            return np.concatenate(
                [res[c]["out"] for c in range(N_CORES)], axis=0
            ).astype(np.float32)
        except Exception as e:  # noqa: BLE001
            last_err = e
    raise last_err


# revision 32
# speedup vs baseline: 1.0102x; 1.0011x over previous
"""Trainium2 Bass kernel for nn_DeltaEncoderBlock.

Reference semantics (all fp32):
    x: [64, 9, 14, 384] -> x_flat [64, 126, 384]
    delta[t] = x[t] - x[t-1]  (delta[0] = x[0])        (temporal delta)
    w = g * v / ||v||_row                               (weight norm, [1024, 126])
    z = einsum('oi,bit->tbo', w, delta)                 (synaptic input)
    scan over t:  cur = 0.75*cur + z_t
                  vol = 0.97*vol + cur
                  s   = (vol >= 1)
                  vol = vol * (1 - s)                   (hard reset)
    out: spikes [64, 1024, 384]

Sharding: data-parallel over batch across 8 NeuronCores (8 batches/core).

Per-core kernel:
  - z via PE fp32 matmuls (K=126), o in 8 chunks of 128 partitions,
    weight-norm scale applied in the PSUM->SBUF copy on ScalarE.
  - cur via DVE tensor_tensor_scan (linear recurrence along t).
  - vol/spike loop: 2 fused scalar_tensor_tensor DVE ops per step;
    spike = Relu(Sign(vol_pre - 1)) on ScalarE (Sign per step, Relu per
    48-step block), DMA'd out per block.
"""

import numpy as np

import concourse.bacc as bacc
import concourse.tile as tile
from concourse import mybir
from concourse.bass_utils import run_bass_kernel_spmd
from concourse.masks import make_identity

N_CORES = 8
B, C, H, T = 64, 9, 14, 384
I = C * H  # 126
O = 1024
BL = B // N_CORES  # 8 batches per core
NCH = O // 128  # 8 o-chunks of 128
TBLK = 64  # t-block: matmul window, z tile span, and spike staging block
NTB = T // TBLK  # 6
F32 = mybir.dt.float32
U8 = mybir.dt.uint8

CURRENT_DECAY = 0.25
VOLTAGE_DECAY = 0.03

# fp32r streams fp32 through the PE at bf16 rate (4x faster than plain fp32
# matmul); numerics differ slightly from fp32 — gated on a HW accuracy check.
MM_F32R = False


def _body(tc, x, v, g, out):
    nc = tc.nc
    Alu = mybir.AluOpType
    Act = mybir.ActivationFunctionType

    import contextlib

    with contextlib.ExitStack() as ctx:
        consts = ctx.enter_context(tc.tile_pool(name="consts", bufs=1))
        big = ctx.enter_context(tc.tile_pool(name="big", bufs=1))
        wp = ctx.enter_context(tc.tile_pool(name="wp", bufs=1))
        psT = ctx.enter_context(tc.tile_pool(name="psT", bufs=2, space="PSUM"))
        psZ = ctx.enter_context(tc.tile_pool(name="psZ", bufs=6, space="PSUM"))
        pvolS = ctx.enter_context(tc.tile_pool(name="pvolS", bufs=8))
        pstage = ctx.enter_context(tc.tile_pool(name="pstage", bufs=2))

        # ---- load x per batch, pipelined with delta + cur-delta scan ----
        # cur-delta: scan the 0.75 recurrence on delta before the matmul;
        # W.(scan delta) == scan (W.delta) by linearity.
        decD = consts.tile([I, 1], F32)
        nc.vector.memset(decD[:], 1.0 - CURRENT_DECAY)
        xs = big.tile([I, BL * T], F32)
        x3 = xs[:].rearrange("p (b t) -> p b t", b=BL)
        delta = big.tile([I, BL * T], F32)
        d3 = delta[:].rearrange("p (b t) -> p b t", b=BL)
        TH = 2 * TBLK  # scan in halves: first half unblocks early matmuls
        for b in range(BL):
            nc.sync.dma_start(x3[:, b, :], x[b].rearrange("i t -> i t"))
            nc.vector.tensor_copy(d3[:, b, 0:1], x3[:, b, 0:1])
            nc.vector.tensor_tensor(
                out=d3[:, b, 1:TH], in0=x3[:, b, 1:TH], in1=x3[:, b, 0 : TH - 1],
                op=Alu.subtract,
            )
            seg = delta[:, b * T : b * T + TH]
            nc.vector.tensor_tensor_scan(
                seg, decD[:].to_broadcast([I, TH]), seg, 0.0, Alu.mult, Alu.add
            )
        for b in range(BL):
            nc.vector.tensor_tensor(
                out=d3[:, b, TH:T], in0=x3[:, b, TH:T], in1=x3[:, b, TH - 1 : T - 1],
                op=Alu.subtract,
            )
            seg = delta[:, b * T + TH : (b + 1) * T]
            carry = delta[:, b * T + TH - 1 : b * T + TH]
            nc.vector.tensor_tensor_scan(
                seg, decD[:].to_broadcast([I, T - TH]), seg, carry,
                Alu.mult, Alu.add,
            )

        # ---- weights: v natural layout, norms, transpose ----
        vt = wp.tile([128, NCH * I], F32)
        vt3 = vt[:].rearrange("p (c i) -> p c i", c=NCH)
        nc.sync.dma_start(vt3, v.rearrange("(c p) i -> p c i", p=128))

        gt = wp.tile([128, NCH], F32)
        nc.sync.dma_start(gt[:], g.rearrange("(c p) -> p c", p=128))

        sq = wp.tile([128, I], F32)  # scratch for Square
        ss = wp.tile([128, NCH], F32)  # row sum-of-squares
        for c in range(NCH):
            nc.scalar.activation(
                sq[:], vt3[:, c, :], Act.Square, accum_out=ss[:, c : c + 1]
            )
        inv = wp.tile([128, NCH], F32)
        nc.vector.reciprocal(inv[:], ss[:])
        rs = wp.tile([128, NCH], F32)
        nc.scalar.sqrt(rs[:], inv[:])  # rsqrt(sum v^2)
        scale = wp.tile([128, NCH], F32)
        nc.vector.tensor_tensor(out=scale[:], in0=rs[:], in1=gt[:], op=Alu.mult)

        ident = consts.tile([128, 128], F32)
        make_identity(nc, ident[:])

        wT = []  # per-chunk [126, 128] tiles of v^T
        for c in range(NCH):
            pt = psT.tile([I, 128], F32)
            nc.tensor.transpose(pt[:], vt3[:, c, :], ident[:])
            wc = wp.tile([I, 128], F32, tag=f"wT{c}")
            nc.scalar.copy(wc[:], pt[:])
            wT.append(wc)

        # ---- cur = (v^T . cur-delta), scaled by g/||v|| on the PSUM->SBUF
        # copy. One z tile per t-block of TBLK steps, layout [p, (c b tl)],
        # so the vol loop starts after the first t-block's matmuls and the
        # rest of the matmul phase hides under the loop. Matmul windows
        # enumerate (tl, b) columns via a strided rhs AP on delta. ----
        dly = delta[:].rearrange("p (b t) -> p t b", b=BL)  # [126, T, BL]
        ztiles = []
        for tb in range(NTB):
            zt = big.tile([128, NCH * BL * TBLK], F32, tag=f"z{tb}")
            ztiles.append(zt)
            # memory layout (c, b, tl); dims permuted to enumerate (tl, b)
            zv = zt[:].rearrange("p (c b tl) -> p c tl b", c=NCH, b=BL)
            for c in range(NCH):
                ps = psZ.tile([128, BL * TBLK], F32)
                mm_lhs = wT[c][:]
                mm_rhs = dly[:, tb * TBLK : (tb + 1) * TBLK, :]
                if MM_F32R:
                    mm_lhs = mm_lhs.bitcast(mybir.dt.float32r)
                    mm_rhs = mm_rhs.bitcast(mybir.dt.float32r)
                nc.tensor.matmul(
                    ps[:], lhsT=mm_lhs, rhs=mm_rhs, start=True, stop=True
                )
                # psum cols are (tl, b); write them to z at (b*TBLK + tl)
                nc.scalar.activation(
                    zv[:, c, :, :],
                    ps[:].rearrange("p (tl b) -> p tl b", b=BL),
                    Act.Copy,
                    scale=scale[:, c : c + 1],
                )

        # ---- vol loop: vol_pre overwrites the cur column of z in place.
        # DVE-only; no cross-engine sync inside the loop. ----
        neg1 = consts.tile([128, 1], F32)
        nc.vector.memset(neg1[:], -1.0)
        volS = None
        vdec = 1.0 - VOLTAGE_DECAY

        out_r = out.rearrange("b (c p) t -> p c b t", c=NCH)
        for t in range(T):
            tb, tl = divmod(t, TBLK)
            zc = ztiles[tb][:].rearrange(
                "p (c b tl) -> p c b tl", c=NCH, b=BL
            )
            # vol_pre = vdec * vol + cur_t   (written over cur_t).
            # t=0: vol_pre = cur_0 is already in place — skip the op.
            if t > 0:
                nc.vector.scalar_tensor_tensor(
                    zc[:, :, :, tl],
                    volS[:].rearrange("p (c b) -> p c b", c=NCH),
                    vdec,
                    zc[:, :, :, tl],
                    Alu.mult,
                    Alu.add,
                )
            # vol = (vol_pre < 1) * vol_pre   (hard reset); the state after
            # the last step is never consumed — skip it.
            if t < T - 1:
                volS = pvolS.tile([128, NCH * BL], F32, tag="volS")
                nc.vector.scalar_tensor_tensor(
                    volS[:].rearrange("p (c b) -> p c b", c=NCH),
                    zc[:, :, :, tl],
                    1.0,
                    zc[:, :, :, tl],
                    Alu.is_lt,
                    Alu.mult,
                )
            # spikes on ACT (off the DVE path): Sign -> in-place Relu ->
            # DMA out, staging (c, b, tl). Block tb-1 is extracted at the
            # START of block tb (its data is complete and the ACT ops are
            # immediately ready); the final block flushes in quarters as
            # its columns finish so only the last quarter sits on the tail.
            flush = []
            if tb >= 1 and tl == 0:
                flush = [(tb - 1, 0, TBLK)]
            if tb == NTB - 1 and (tl + 1) % (TBLK // 4) == 0:
                q = (tl + 1) // (TBLK // 4) - 1
                flush += [(tb, q * (TBLK // 4), (q + 1) * (TBLK // 4))]
            for ftb, lo, hi in flush:
                w = hi - lo
                zcf = ztiles[ftb][:].rearrange(
                    "p (c b tl) -> p c b tl", c=NCH, b=BL
                )
                ostage = pstage.tile([128, 64 * w], U8, tag=f"os{w}")
                o3 = ostage[:].rearrange(
                    "p (c b tl) -> p c b tl", c=NCH, b=BL
                )
                # spikes are exact 0/1: Sign then Relu (which also narrows
                # to uint8 -> 4x fewer DMA bytes; host widens). A DVE
                # is_ge->uint8 shortcut for the final quarter matched in
                # CoreSim but was WRONG on hardware — keep ACT.
                sstage = pstage.tile([128, 64 * w], F32, tag=f"ss{w}")
                s3 = sstage[:].rearrange(
                    "p (c b tl) -> p c b tl", c=NCH, b=BL
                )
                nc.scalar.activation(
                    s3, zcf[:, :, :, lo:hi], Act.Sign, bias=neg1[:]
                )
                nc.scalar.activation(o3, s3, Act.Relu)
                for c in range(NCH):
                    nc.sync.dma_start(
                        out_r[:, c, :, ftb * TBLK + lo : ftb * TBLK + hi],
                        o3[:, c, :, :],
                    )


_CACHE = {}


def _build():
    if "nc" in _CACHE:
        return _CACHE["nc"]
    nc = bacc.Bacc(
        "TRN2", target_bir_lowering=False, debug=False, num_devices=N_CORES
    )
    x = nc.dram_tensor("x", [BL, I, T], F32, kind="ExternalInput").ap()
    v = nc.dram_tensor("v", [O, I], F32, kind="ExternalInput").ap()
    g = nc.dram_tensor("g", [O], F32, kind="ExternalInput").ap()
    out = nc.dram_tensor("out", [BL, O, T], U8, kind="ExternalOutput").ap()
    with tile.TileContext(nc) as tc:
        _body(tc, x, v, g, out)
    nc.compile()
    _CACHE["nc"] = nc
    return nc


def make_in_maps(x, v_weight, g):
    xr = np.ascontiguousarray(x.reshape(B, I, T))
    v_weight = np.ascontiguousarray(v_weight)
    g = np.ascontiguousarray(g)
    return [
        {
            "x": np.ascontiguousarray(xr[c * BL : (c + 1) * BL]),
            "v": v_weight,
            "g": g,
        }
        for c in range(N_CORES)
    ]


def kernel(x, v_weight, g):
    nc = _build()
    in_maps = make_in_maps(
        np.asarray(x, dtype=np.float32),
        np.asarray(v_weight, dtype=np.float32),
        np.asarray(g, dtype=np.float32),
    )
    last_err = None
    for _attempt in range(3):  # retry: a prior tenant can leave a core wedged
        try:
            res = run_bass_kernel_spmd(nc, in_maps, list(range(N_CORES))).results
            return np.concatenate(
                [res[c]["out"] for c in range(N_CORES)], axis=0
            ).astype(np.float32)
        except Exception as e:  # noqa: BLE001
            last_err = e
    raise last_err


# revision 33
# speedup vs baseline: 1.0284x; 1.0180x over previous
"""Trainium2 Bass kernel for nn_DeltaEncoderBlock.

Reference semantics (all fp32):
    x: [64, 9, 14, 384] -> x_flat [64, 126, 384]
    delta[t] = x[t] - x[t-1]  (delta[0] = x[0])        (temporal delta)
    w = g * v / ||v||_row                               (weight norm, [1024, 126])
    z = einsum('oi,bit->tbo', w, delta)                 (synaptic input)
    scan over t:  cur = 0.75*cur + z_t
                  vol = 0.97*vol + cur
                  s   = (vol >= 1)
                  vol = vol * (1 - s)                   (hard reset)
    out: spikes [64, 1024, 384]

Sharding: data-parallel over batch across 8 NeuronCores (8 batches/core).

Per-core kernel:
  - z via PE fp32 matmuls (K=126), o in 8 chunks of 128 partitions,
    weight-norm scale applied in the PSUM->SBUF copy on ScalarE.
  - cur via DVE tensor_tensor_scan (linear recurrence along t).
  - vol/spike loop: 2 fused scalar_tensor_tensor DVE ops per step;
    spike = Relu(Sign(vol_pre - 1)) on ScalarE (Sign per step, Relu per
    48-step block), DMA'd out per block.
"""

import numpy as np

import concourse.bacc as bacc
import concourse.tile as tile
from concourse import mybir
from concourse.bass_utils import run_bass_kernel_spmd
from concourse.masks import make_identity

N_CORES = 8
B, C, H, T = 64, 9, 14, 384
I = C * H  # 126
O = 1024
BL = B // N_CORES  # 8 batches per core
NCH = O // 128  # 8 o-chunks of 128
TBLK = 64  # t-block: matmul window, z tile span, and spike staging block
NTB = T // TBLK  # 6
F32 = mybir.dt.float32
U8 = mybir.dt.uint8

CURRENT_DECAY = 0.25
VOLTAGE_DECAY = 0.03

# fp32r streams fp32 through the PE at bf16 rate (4x faster than plain fp32
# matmul); numerics differ slightly from fp32 — gated on a HW accuracy check.
MM_F32R = False


def _body(tc, x, v, g, out):
    nc = tc.nc
    Alu = mybir.AluOpType
    Act = mybir.ActivationFunctionType

    import contextlib

    with contextlib.ExitStack() as ctx:
        consts = ctx.enter_context(tc.tile_pool(name="consts", bufs=1))
        big = ctx.enter_context(tc.tile_pool(name="big", bufs=1))
        wp = ctx.enter_context(tc.tile_pool(name="wp", bufs=1))
        psT = ctx.enter_context(tc.tile_pool(name="psT", bufs=2, space="PSUM"))
        psZ = ctx.enter_context(tc.tile_pool(name="psZ", bufs=6, space="PSUM"))
        pvolS = ctx.enter_context(tc.tile_pool(name="pvolS", bufs=8))
        pstage = ctx.enter_context(tc.tile_pool(name="pstage", bufs=2))

        # ---- load x per batch, pipelined with delta + cur-delta scan ----
        # cur-delta: scan the 0.75 recurrence on delta before the matmul;
        # W.(scan delta) == scan (W.delta) by linearity.
        decD = consts.tile([I, 1], F32)
        nc.vector.memset(decD[:], 1.0 - CURRENT_DECAY)
        xs = big.tile([I, BL * T], F32)
        x3 = xs[:].rearrange("p (b t) -> p b t", b=BL)
        delta = big.tile([I, BL * T], F32)
        d3 = delta[:].rearrange("p (b t) -> p b t", b=BL)
        TH = 2 * TBLK  # scan in halves: first half unblocks early matmuls
        for b in range(BL):
            nc.sync.dma_start(x3[:, b, :], x[b].rearrange("i t -> i t"))
            nc.vector.tensor_copy(d3[:, b, 0:1], x3[:, b, 0:1])
            nc.vector.tensor_tensor(
                out=d3[:, b, 1:TH], in0=x3[:, b, 1:TH], in1=x3[:, b, 0 : TH - 1],
                op=Alu.subtract,
            )
            seg = delta[:, b * T : b * T + TH]
            nc.vector.tensor_tensor_scan(
                seg, decD[:].to_broadcast([I, TH]), seg, 0.0, Alu.mult, Alu.add
            )
        for b in range(BL):
            nc.vector.tensor_tensor(
                out=d3[:, b, TH:T], in0=x3[:, b, TH:T], in1=x3[:, b, TH - 1 : T - 1],
                op=Alu.subtract,
            )
            seg = delta[:, b * T + TH : (b + 1) * T]
            carry = delta[:, b * T + TH - 1 : b * T + TH]
            nc.vector.tensor_tensor_scan(
                seg, decD[:].to_broadcast([I, T - TH]), seg, carry,
                Alu.mult, Alu.add,
            )

        # ---- weights: v natural layout, norms, transpose ----
        vt = wp.tile([128, NCH * I], F32)
        vt3 = vt[:].rearrange("p (c i) -> p c i", c=NCH)
        nc.sync.dma_start(vt3, v.rearrange("(c p) i -> p c i", p=128))

        gt = wp.tile([128, NCH], F32)
        nc.sync.dma_start(gt[:], g.rearrange("(c p) -> p c", p=128))

        sq = wp.tile([128, I], F32)  # scratch for Square
        ss = wp.tile([128, NCH], F32)  # row sum-of-squares
        for c in range(NCH):
            nc.scalar.activation(
                sq[:], vt3[:, c, :], Act.Square, accum_out=ss[:, c : c + 1]
            )
        inv = wp.tile([128, NCH], F32)
        nc.vector.reciprocal(inv[:], ss[:])
        rs = wp.tile([128, NCH], F32)
        nc.scalar.sqrt(rs[:], inv[:])  # rsqrt(sum v^2)
        scale = wp.tile([128, NCH], F32)
        nc.vector.tensor_tensor(out=scale[:], in0=rs[:], in1=gt[:], op=Alu.mult)

        ident = consts.tile([128, 128], F32)
        make_identity(nc, ident[:])

        # PE HAM warm-up: dummy matmuls during the input DMA so the real
        # matmuls run at 2.4GHz from the start (HAM un-throttles after
        # ~3.4us of sustained PE activity). Results are never read; real
        # matmuls use start=True so the shared PSUM slots are reset.
        for _ in range(16):
            wps = psZ.tile([128, BL * TBLK], F32, tag="ps")
            nc.tensor.matmul(
                wps[:, 0:128], lhsT=ident[:], rhs=ident[:],
                start=True, stop=True,
            )

        wT = []  # per-chunk [126, 128] tiles of v^T
        for c in range(NCH):
            pt = psT.tile([I, 128], F32)
            nc.tensor.transpose(pt[:], vt3[:, c, :], ident[:])
            wc = wp.tile([I, 128], F32, tag=f"wT{c}")
            nc.scalar.copy(wc[:], pt[:])
            wT.append(wc)

        # ---- cur = (v^T . cur-delta), scaled by g/||v|| on the PSUM->SBUF
        # copy. One z tile per t-block of TBLK steps, layout [p, (c b tl)],
        # so the vol loop starts after the first t-block's matmuls and the
        # rest of the matmul phase hides under the loop. Matmul windows
        # enumerate (tl, b) columns via a strided rhs AP on delta. ----
        dly = delta[:].rearrange("p (b t) -> p t b", b=BL)  # [126, T, BL]
        ztiles = []
        for tb in range(NTB):
            zt = big.tile([128, NCH * BL * TBLK], F32, tag=f"z{tb}")
            ztiles.append(zt)
            # memory layout (c, b, tl); dims permuted to enumerate (tl, b)
            zv = zt[:].rearrange("p (c b tl) -> p c tl b", c=NCH, b=BL)
            for c in range(NCH):
                ps = psZ.tile([128, BL * TBLK], F32)
                mm_lhs = wT[c][:]
                mm_rhs = dly[:, tb * TBLK : (tb + 1) * TBLK, :]
                if MM_F32R:
                    mm_lhs = mm_lhs.bitcast(mybir.dt.float32r)
                    mm_rhs = mm_rhs.bitcast(mybir.dt.float32r)
                nc.tensor.matmul(
                    ps[:], lhsT=mm_lhs, rhs=mm_rhs, start=True, stop=True
                )
                # psum cols are (tl, b); write them to z at (b*TBLK + tl)
                nc.scalar.activation(
                    zv[:, c, :, :],
                    ps[:].rearrange("p (tl b) -> p tl b", b=BL),
                    Act.Copy,
                    scale=scale[:, c : c + 1],
                )

        # ---- vol loop: vol_pre overwrites the cur column of z in place.
        # DVE-only; no cross-engine sync inside the loop. ----
        neg1 = consts.tile([128, 1], F32)
        nc.vector.memset(neg1[:], -1.0)
        volS = None
        vdec = 1.0 - VOLTAGE_DECAY

        out_r = out.rearrange("b (c p) t -> p c b t", c=NCH)
        for t in range(T):
            tb, tl = divmod(t, TBLK)
            zc = ztiles[tb][:].rearrange(
                "p (c b tl) -> p c b tl", c=NCH, b=BL
            )
            # vol_pre = vdec * vol + cur_t   (written over cur_t).
            # t=0: vol_pre = cur_0 is already in place — skip the op.
            if t > 0:
                nc.vector.scalar_tensor_tensor(
                    zc[:, :, :, tl],
                    volS[:].rearrange("p (c b) -> p c b", c=NCH),
                    vdec,
                    zc[:, :, :, tl],
                    Alu.mult,
                    Alu.add,
                )
            # vol = (vol_pre < 1) * vol_pre   (hard reset); the state after
            # the last step is never consumed — skip it.
            if t < T - 1:
                volS = pvolS.tile([128, NCH * BL], F32, tag="volS")
                nc.vector.scalar_tensor_tensor(
                    volS[:].rearrange("p (c b) -> p c b", c=NCH),
                    zc[:, :, :, tl],
                    1.0,
                    zc[:, :, :, tl],
                    Alu.is_lt,
                    Alu.mult,
                )
            # spikes on ACT (off the DVE path): Sign -> in-place Relu ->
            # DMA out, staging (c, b, tl). Block tb-1 is extracted at the
            # START of block tb (its data is complete and the ACT ops are
            # immediately ready); the final block flushes in quarters as
            # its columns finish so only the last quarter sits on the tail.
            flush = []
            if tb >= 1 and tl == 0:
                flush = [(tb - 1, 0, TBLK)]
            if tb == NTB - 1 and (tl + 1) % (TBLK // 4) == 0:
                q = (tl + 1) // (TBLK // 4) - 1
                flush += [(tb, q * (TBLK // 4), (q + 1) * (TBLK // 4))]
            for ftb, lo, hi in flush:
                w = hi - lo
                zcf = ztiles[ftb][:].rearrange(
                    "p (c b tl) -> p c b tl", c=NCH, b=BL
                )
                ostage = pstage.tile([128, 64 * w], U8, tag=f"os{w}")
                o3 = ostage[:].rearrange(
                    "p (c b tl) -> p c b tl", c=NCH, b=BL
                )
                # spikes are exact 0/1: Sign then Relu (which also narrows
                # to uint8 -> 4x fewer DMA bytes; host widens). A DVE
                # is_ge->uint8 shortcut for the final quarter matched in
                # CoreSim but was WRONG on hardware — keep ACT.
                sstage = pstage.tile([128, 64 * w], F32, tag=f"ss{w}")
                s3 = sstage[:].rearrange(
                    "p (c b tl) -> p c b tl", c=NCH, b=BL
                )
                nc.scalar.activation(
                    s3, zcf[:, :, :, lo:hi], Act.Sign, bias=neg1[:]
                )
                nc.scalar.activation(o3, s3, Act.Relu)
                for c in range(NCH):
                    nc.sync.dma_start(
                        out_r[:, c, :, ftb * TBLK + lo : ftb * TBLK + hi],
                        o3[:, c, :, :],
                    )


_CACHE = {}


def _build():
    if "nc" in _CACHE:
        return _CACHE["nc"]
    nc = bacc.Bacc(
        "TRN2", target_bir_lowering=False, debug=False, num_devices=N_CORES
    )
    x = nc.dram_tensor("x", [BL, I, T], F32, kind="ExternalInput").ap()
    v = nc.dram_tensor("v", [O, I], F32, kind="ExternalInput").ap()
    g = nc.dram_tensor("g", [O], F32, kind="ExternalInput").ap()
    out = nc.dram_tensor("out", [BL, O, T], U8, kind="ExternalOutput").ap()
    with tile.TileContext(nc) as tc:
        _body(tc, x, v, g, out)
    nc.compile()
    _CACHE["nc"] = nc
    return nc


def make_in_maps(x, v_weight, g):
    xr = np.ascontiguousarray(x.reshape(B, I, T))
    v_weight = np.ascontiguousarray(v_weight)
    g = np.ascontiguousarray(g)
    return [
        {
            "x": np.ascontiguousarray(xr[c * BL : (c + 1) * BL]),
            "v": v_weight,
            "g": g,
        }
        for c in range(N_CORES)
    ]


def kernel(x, v_weight, g):
    nc = _build()
    in_maps = make_in_maps(
        np.asarray(x, dtype=np.float32),
        np.asarray(v_weight, dtype=np.float32),
        np.asarray(g, dtype=np.float32),
    )
    last_err = None
    for _attempt in range(3):  # retry: a prior tenant can leave a core wedged
        try:
            res = run_bass_kernel_spmd(nc, in_maps, list(range(N_CORES))).results
            return np.concatenate(
                [res[c]["out"] for c in range(N_CORES)], axis=0
            ).astype(np.float32)
        except Exception as e:  # noqa: BLE001
            last_err = e
    raise last_err


# revision 36
# speedup vs baseline: 1.0446x; 1.0158x over previous
"""Trainium2 Bass kernel for nn_DeltaEncoderBlock.

Reference semantics (all fp32):
    x: [64, 9, 14, 384] -> x_flat [64, 126, 384]
    delta[t] = x[t] - x[t-1]  (delta[0] = x[0])        (temporal delta)
    w = g * v / ||v||_row                               (weight norm, [1024, 126])
    z = einsum('oi,bit->tbo', w, delta)                 (synaptic input)
    scan over t:  cur = 0.75*cur + z_t
                  vol = 0.97*vol + cur
                  s   = (vol >= 1)
                  vol = vol * (1 - s)                   (hard reset)
    out: spikes [64, 1024, 384]

Sharding: data-parallel over batch across 8 NeuronCores (8 batches/core).

Per-core kernel:
  - z via PE fp32 matmuls (K=126), o in 8 chunks of 128 partitions,
    weight-norm scale applied in the PSUM->SBUF copy on ScalarE.
  - cur via DVE tensor_tensor_scan (linear recurrence along t).
  - vol/spike loop: 2 fused scalar_tensor_tensor DVE ops per step;
    spike = Relu(Sign(vol_pre - 1)) on ScalarE (Sign per step, Relu per
    48-step block), DMA'd out per block.
"""

import numpy as np

import concourse.bacc as bacc
import concourse.tile as tile
from concourse import mybir
from concourse.bass_utils import run_bass_kernel_spmd
from concourse.masks import make_identity

N_CORES = 8
B, C, H, T = 64, 9, 14, 384
I = C * H  # 126
O = 1024
BL = B // N_CORES  # 8 batches per core
NCH = O // 128  # 8 o-chunks of 128
TBLK = 64  # t-block: matmul window, z tile span, and spike staging block
NTB = T // TBLK  # 6
F32 = mybir.dt.float32
U8 = mybir.dt.uint8

CURRENT_DECAY = 0.25
VOLTAGE_DECAY = 0.03

# fp32r streams fp32 through the PE at bf16 rate (4x faster than plain fp32
# matmul); numerics differ slightly from fp32 — gated on a HW accuracy check.
MM_F32R = False


def _body(tc, x, v, g, out):
    nc = tc.nc
    Alu = mybir.AluOpType
    Act = mybir.ActivationFunctionType

    import contextlib

    with contextlib.ExitStack() as ctx:
        consts = ctx.enter_context(tc.tile_pool(name="consts", bufs=1))
        big = ctx.enter_context(tc.tile_pool(name="big", bufs=1))
        wp = ctx.enter_context(tc.tile_pool(name="wp", bufs=1))
        psT = ctx.enter_context(tc.tile_pool(name="psT", bufs=2, space="PSUM"))
        psZ = ctx.enter_context(tc.tile_pool(name="psZ", bufs=6, space="PSUM"))
        pvolS = ctx.enter_context(tc.tile_pool(name="pvolS", bufs=8))
        pstage = ctx.enter_context(tc.tile_pool(name="pstage", bufs=2))

        # ---- front-load the weight path: the norm chain (v DMA -> Squares
        # -> rsqrt -> scale) gates the first z copies and thus the loop
        # start, so its DMAs go on the queue FIRST and both ACT function
        # tables (Square/... and Sqrt) are loaded by dummy ops at t~0. ----
        decD = consts.tile([I, 1], F32)
        nc.vector.memset(decD[:], 1.0 - CURRENT_DECAY)
        neg1 = consts.tile([128, 1], F32)
        nc.vector.memset(neg1[:], -1.0)
        actwarm = consts.tile([128, 1], F32)
        nc.scalar.activation(actwarm[:], neg1[:], Act.Square)
        nc.scalar.activation(actwarm[:], actwarm[:], Act.Sqrt)

        vt = wp.tile([128, NCH * I], F32)
        vt3 = vt[:].rearrange("p (c i) -> p c i", c=NCH)
        nc.sync.dma_start(vt3, v.rearrange("(c p) i -> p c i", p=128))
        gt = wp.tile([128, NCH], F32)
        nc.sync.dma_start(gt[:], g.rearrange("(c p) -> p c", p=128))

        sq = wp.tile([128, I], F32)  # scratch for Square
        ss = wp.tile([128, NCH], F32)  # row sum-of-squares
        for c in range(NCH):
            nc.scalar.activation(
                sq[:], vt3[:, c, :], Act.Square, accum_out=ss[:, c : c + 1]
            )
        inv = wp.tile([128, NCH], F32)
        nc.vector.reciprocal(inv[:], ss[:])
        rs = wp.tile([128, NCH], F32)
        nc.scalar.sqrt(rs[:], inv[:])  # rsqrt(sum v^2)
        scale = wp.tile([128, NCH], F32)
        nc.vector.tensor_tensor(out=scale[:], in0=rs[:], in1=gt[:], op=Alu.mult)

        # ---- load x per batch, pipelined with delta + cur-delta scan ----
        # cur-delta: scan the 0.75 recurrence on delta before the matmul;
        # W.(scan delta) == scan (W.delta) by linearity.
        xs = big.tile([I, BL * T], F32)
        x3 = xs[:].rearrange("p (b t) -> p b t", b=BL)
        delta = big.tile([I, BL * T], F32)
        d3 = delta[:].rearrange("p (b t) -> p b t", b=BL)
        TH = 2 * TBLK  # scan in halves: first half unblocks early matmuls
        for b in range(BL):
            nc.sync.dma_start(x3[:, b, :], x[b].rearrange("i t -> i t"))
            nc.vector.tensor_copy(d3[:, b, 0:1], x3[:, b, 0:1])
            nc.vector.tensor_tensor(
                out=d3[:, b, 1:TH], in0=x3[:, b, 1:TH], in1=x3[:, b, 0 : TH - 1],
                op=Alu.subtract,
            )
            seg = delta[:, b * T : b * T + TH]
            nc.vector.tensor_tensor_scan(
                seg, decD[:].to_broadcast([I, TH]), seg, 0.0, Alu.mult, Alu.add
            )
        for b in range(BL):
            nc.vector.tensor_tensor(
                out=d3[:, b, TH:T], in0=x3[:, b, TH:T], in1=x3[:, b, TH - 1 : T - 1],
                op=Alu.subtract,
            )
            seg = delta[:, b * T + TH : (b + 1) * T]
            carry = delta[:, b * T + TH - 1 : b * T + TH]
            nc.vector.tensor_tensor_scan(
                seg, decD[:].to_broadcast([I, T - TH]), seg, carry,
                Alu.mult, Alu.add,
            )

        ident = consts.tile([128, 128], F32)
        make_identity(nc, ident[:])

        # PE HAM warm-up: dummy matmuls during the input DMA so the real
        # matmuls run at 2.4GHz from the start (HAM un-throttles after
        # ~3.4us of sustained PE activity). Results are never read; real
        # matmuls use start=True so the shared PSUM slots are reset.
        for _ in range(16):
            wps = psZ.tile([128, BL * TBLK], F32, tag="ps")
            nc.tensor.matmul(
                wps[:, 0:128], lhsT=ident[:], rhs=ident[:],
                start=True, stop=True,
            )

        wT = []  # per-chunk [126, 128] tiles of v^T
        for c in range(NCH):
            pt = psT.tile([I, 128], F32)
            nc.tensor.transpose(pt[:], vt3[:, c, :], ident[:])
            wc = wp.tile([I, 128], F32, tag=f"wT{c}")
            nc.scalar.copy(wc[:], pt[:])
            wT.append(wc)

        # ---- cur = (v^T . cur-delta), scaled by g/||v|| on the PSUM->SBUF
        # copy. One z tile per t-block of TBLK steps, layout [p, (c b tl)],
        # so the vol loop starts after the first t-block's matmuls and the
        # rest of the matmul phase hides under the loop. Matmul windows
        # enumerate (tl, b) columns via a strided rhs AP on delta. ----
        dly = delta[:].rearrange("p (b t) -> p t b", b=BL)  # [126, T, BL]
        ztiles = []
        for tb in range(NTB):
            zt = big.tile([128, NCH * BL * TBLK], F32, tag=f"z{tb}")
            ztiles.append(zt)
            # memory layout (c, b, tl); dims permuted to enumerate (tl, b)
            zv = zt[:].rearrange("p (c b tl) -> p c tl b", c=NCH, b=BL)
            for c in range(NCH):
                ps = psZ.tile([128, BL * TBLK], F32)
                mm_lhs = wT[c][:]
                mm_rhs = dly[:, tb * TBLK : (tb + 1) * TBLK, :]
                if MM_F32R:
                    mm_lhs = mm_lhs.bitcast(mybir.dt.float32r)
                    mm_rhs = mm_rhs.bitcast(mybir.dt.float32r)
                nc.tensor.matmul(
                    ps[:], lhsT=mm_lhs, rhs=mm_rhs, start=True, stop=True
                )
                # psum cols are (tl, b); write them to z at (b*TBLK + tl)
                nc.scalar.activation(
                    zv[:, c, :, :],
                    ps[:].rearrange("p (tl b) -> p tl b", b=BL),
                    Act.Copy,
                    scale=scale[:, c : c + 1],
                )

        # ---- vol loop: vol_pre overwrites the cur column of z in place.
        # DVE-only; no cross-engine sync inside the loop. ----
        volS = None
        vdec = 1.0 - VOLTAGE_DECAY

        out_r = out.rearrange("b (c p) t -> p c b t", c=NCH)
        for t in range(T):
            tb, tl = divmod(t, TBLK)
            zc = ztiles[tb][:].rearrange(
                "p (c b tl) -> p c b tl", c=NCH, b=BL
            )
            # vol_pre = vdec * vol + cur_t   (written over cur_t).
            # t=0: vol_pre = cur_0 is already in place — skip the op.
            if t > 0:
                nc.vector.scalar_tensor_tensor(
                    zc[:, :, :, tl],
                    volS[:].rearrange("p (c b) -> p c b", c=NCH),
                    vdec,
                    zc[:, :, :, tl],
                    Alu.mult,
                    Alu.add,
                )
            # vol = (vol_pre < 1) * vol_pre   (hard reset); the state after
            # the last step is never consumed — skip it.
            if t < T - 1:
                volS = pvolS.tile([128, NCH * BL], F32, tag="volS")
                nc.vector.scalar_tensor_tensor(
                    volS[:].rearrange("p (c b) -> p c b", c=NCH),
                    zc[:, :, :, tl],
                    1.0,
                    zc[:, :, :, tl],
                    Alu.is_lt,
                    Alu.mult,
                )
            # spikes on ACT (off the DVE path): Sign -> in-place Relu ->
            # DMA out, staging (c, b, tl). Block tb-1 is extracted at the
            # START of block tb (its data is complete and the ACT ops are
            # immediately ready); the final block flushes in quarters as
            # its columns finish so only the last quarter sits on the tail.
            flush = []
            if tb >= 1 and tl == 0:
                flush = [(tb - 1, 0, TBLK)]
            if tb == NTB - 1 and (tl + 1) % (TBLK // 4) == 0:
                q = (tl + 1) // (TBLK // 4) - 1
                flush += [(tb, q * (TBLK // 4), (q + 1) * (TBLK // 4))]
            for ftb, lo, hi in flush:
                w = hi - lo
                zcf = ztiles[ftb][:].rearrange(
                    "p (c b tl) -> p c b tl", c=NCH, b=BL
                )
                ostage = pstage.tile([128, 64 * w], U8, tag=f"os{w}")
                o3 = ostage[:].rearrange(
                    "p (c b tl) -> p c b tl", c=NCH, b=BL
                )
                # spikes are exact 0/1: Sign then Relu (which also narrows
                # to uint8 -> 4x fewer DMA bytes; host widens). A DVE
                # is_ge->uint8 shortcut for the final quarter matched in
                # CoreSim but was WRONG on hardware — keep ACT.
                sstage = pstage.tile([128, 64 * w], F32, tag=f"ss{w}")
                s3 = sstage[:].rearrange(
                    "p (c b tl) -> p c b tl", c=NCH, b=BL
                )
                nc.scalar.activation(
                    s3, zcf[:, :, :, lo:hi], Act.Sign, bias=neg1[:]
                )
                nc.scalar.activation(o3, s3, Act.Relu)
                for c in range(NCH):
                    nc.sync.dma_start(
                        out_r[:, c, :, ftb * TBLK + lo : ftb * TBLK + hi],
                        o3[:, c, :, :],
                    )


_CACHE = {}


def _build():
    if "nc" in _CACHE:
        return _CACHE["nc"]
    nc = bacc.Bacc(
        "TRN2", target_bir_lowering=False, debug=False, num_devices=N_CORES
    )
    x = nc.dram_tensor("x", [BL, I, T], F32, kind="ExternalInput").ap()
    v = nc.dram_tensor("v", [O, I], F32, kind="ExternalInput").ap()
    g = nc.dram_tensor("g", [O], F32, kind="ExternalInput").ap()
    out = nc.dram_tensor("out", [BL, O, T], U8, kind="ExternalOutput").ap()
    with tile.TileContext(nc) as tc:
        _body(tc, x, v, g, out)
    nc.compile()
    _CACHE["nc"] = nc
    return nc


def make_in_maps(x, v_weight, g):
    xr = np.ascontiguousarray(x.reshape(B, I, T))
    v_weight = np.ascontiguousarray(v_weight)
    g = np.ascontiguousarray(g)
    return [
        {
            "x": np.ascontiguousarray(xr[c * BL : (c + 1) * BL]),
            "v": v_weight,
            "g": g,
        }
        for c in range(N_CORES)
    ]


def kernel(x, v_weight, g):
    nc = _build()
    in_maps = make_in_maps(
        np.asarray(x, dtype=np.float32),
        np.asarray(v_weight, dtype=np.float32),
        np.asarray(g, dtype=np.float32),
    )
    last_err = None
    for _attempt in range(3):  # retry: a prior tenant can leave a core wedged
        try:
            res = run_bass_kernel_spmd(nc, in_maps, list(range(N_CORES))).results
            return np.concatenate(
                [res[c]["out"] for c in range(N_CORES)], axis=0
            ).astype(np.float32)
        except Exception as e:  # noqa: BLE001
            last_err = e
    raise last_err


# revision 37
# speedup vs baseline: 1.0624x; 1.0170x over previous
"""Trainium2 Bass kernel for nn_DeltaEncoderBlock.

Reference semantics (all fp32):
    x: [64, 9, 14, 384] -> x_flat [64, 126, 384]
    delta[t] = x[t] - x[t-1]  (delta[0] = x[0])        (temporal delta)
    w = g * v / ||v||_row                               (weight norm, [1024, 126])
    z = einsum('oi,bit->tbo', w, delta)                 (synaptic input)
    scan over t:  cur = 0.75*cur + z_t
                  vol = 0.97*vol + cur
                  s   = (vol >= 1)
                  vol = vol * (1 - s)                   (hard reset)
    out: spikes [64, 1024, 384]

Sharding: data-parallel over batch across 8 NeuronCores (8 batches/core).

Per-core kernel:
  - z via PE fp32 matmuls (K=126), o in 8 chunks of 128 partitions,
    weight-norm scale applied in the PSUM->SBUF copy on ScalarE.
  - cur via DVE tensor_tensor_scan (linear recurrence along t).
  - vol/spike loop: 2 fused scalar_tensor_tensor DVE ops per step;
    spike = Relu(Sign(vol_pre - 1)) on ScalarE (Sign per step, Relu per
    48-step block), DMA'd out per block.
"""

import numpy as np

import concourse.bacc as bacc
import concourse.tile as tile
from concourse import mybir
from concourse.bass_utils import run_bass_kernel_spmd
from concourse.masks import make_identity

N_CORES = 8
B, C, H, T = 64, 9, 14, 384
I = C * H  # 126
O = 1024
BL = B // N_CORES  # 8 batches per core
NCH = O // 128  # 8 o-chunks of 128
TBLK = 64  # t-block: matmul window, z tile span, and spike staging block
NTB = T // TBLK  # 6
F32 = mybir.dt.float32
U8 = mybir.dt.uint8

CURRENT_DECAY = 0.25
VOLTAGE_DECAY = 0.03

# fp32r streams fp32 through the PE at bf16 rate (4x faster than plain fp32
# matmul); numerics differ slightly from fp32 — gated on a HW accuracy check.
MM_F32R = False


def _body(tc, x, v, g, out):
    nc = tc.nc
    Alu = mybir.AluOpType
    Act = mybir.ActivationFunctionType

    import contextlib

    with contextlib.ExitStack() as ctx:
        consts = ctx.enter_context(tc.tile_pool(name="consts", bufs=1))
        big = ctx.enter_context(tc.tile_pool(name="big", bufs=1))
        wp = ctx.enter_context(tc.tile_pool(name="wp", bufs=1))
        psT = ctx.enter_context(tc.tile_pool(name="psT", bufs=2, space="PSUM"))
        psZ = ctx.enter_context(tc.tile_pool(name="psZ", bufs=6, space="PSUM"))
        pvolS = ctx.enter_context(tc.tile_pool(name="pvolS", bufs=8))
        pstage = ctx.enter_context(tc.tile_pool(name="pstage", bufs=2))

        # ---- front-load the weight path: the norm chain (v DMA -> Squares
        # -> rsqrt -> scale) gates the first z copies and thus the loop
        # start, so its DMAs go on the queue FIRST and both ACT function
        # tables (Square/... and Sqrt) are loaded by dummy ops at t~0. ----
        decD = consts.tile([I, 1], F32)
        nc.vector.memset(decD[:], 1.0 - CURRENT_DECAY)
        neg1 = consts.tile([128, 1], F32)
        nc.vector.memset(neg1[:], -1.0)
        actwarm = consts.tile([128, 1], F32)
        nc.scalar.activation(actwarm[:], neg1[:], Act.Square)
        nc.scalar.activation(actwarm[:], actwarm[:], Act.Sqrt)

        vt = wp.tile([128, NCH * I], F32)
        vt3 = vt[:].rearrange("p (c i) -> p c i", c=NCH)
        nc.sync.dma_start(vt3, v.rearrange("(c p) i -> p c i", p=128))
        gt = wp.tile([128, NCH], F32)
        nc.sync.dma_start(gt[:], g.rearrange("(c p) -> p c", p=128))

        sq = wp.tile([128, I], F32)  # scratch for Square
        ss = wp.tile([128, NCH], F32)  # row sum-of-squares
        for c in range(NCH):
            nc.scalar.activation(
                sq[:], vt3[:, c, :], Act.Square, accum_out=ss[:, c : c + 1]
            )
        inv = wp.tile([128, NCH], F32)
        nc.vector.reciprocal(inv[:], ss[:])
        rs = wp.tile([128, NCH], F32)
        nc.scalar.sqrt(rs[:], inv[:])  # rsqrt(sum v^2)
        scale = wp.tile([128, NCH], F32)
        nc.vector.tensor_tensor(out=scale[:], in0=rs[:], in1=gt[:], op=Alu.mult)

        # ---- load x per batch, pipelined with delta + cur-delta scan ----
        # cur-delta: scan the 0.75 recurrence on delta before the matmul;
        # W.(scan delta) == scan (W.delta) by linearity.
        xs = big.tile([I, BL * T], F32)
        x3 = xs[:].rearrange("p (b t) -> p b t", b=BL)
        delta = big.tile([I, BL * T], F32)
        d3 = delta[:].rearrange("p (b t) -> p b t", b=BL)
        TH = 2 * TBLK  # scan in halves: first half unblocks early matmuls
        for b in range(BL):
            nc.sync.dma_start(x3[:, b, :], x[b].rearrange("i t -> i t"))
            nc.vector.tensor_copy(d3[:, b, 0:1], x3[:, b, 0:1])
            nc.vector.tensor_tensor(
                out=d3[:, b, 1:TH], in0=x3[:, b, 1:TH], in1=x3[:, b, 0 : TH - 1],
                op=Alu.subtract,
            )
            seg = delta[:, b * T : b * T + TH]
            nc.vector.tensor_tensor_scan(
                seg, decD[:].to_broadcast([I, TH]), seg, 0.0, Alu.mult, Alu.add
            )
        for b in range(BL):
            nc.vector.tensor_tensor(
                out=d3[:, b, TH:T], in0=x3[:, b, TH:T], in1=x3[:, b, TH - 1 : T - 1],
                op=Alu.subtract,
            )
            seg = delta[:, b * T + TH : (b + 1) * T]
            carry = delta[:, b * T + TH - 1 : b * T + TH]
            nc.vector.tensor_tensor_scan(
                seg, decD[:].to_broadcast([I, T - TH]), seg, carry,
                Alu.mult, Alu.add,
            )

        ident = consts.tile([128, 128], F32)
        make_identity(nc, ident[:])

        # PE HAM warm-up: dummy matmuls during the input DMA so the real
        # matmuls run at 2.4GHz from the start (HAM un-throttles after
        # ~3.4us of sustained PE activity). Results are never read; real
        # matmuls use start=True so the shared PSUM slots are reset.
        for _ in range(16):
            wps = psZ.tile([128, BL * TBLK], F32, tag="ps")
            nc.tensor.matmul(
                wps[:, 0:128], lhsT=ident[:], rhs=ident[:],
                start=True, stop=True,
            )

        wT = []  # per-chunk [126, 128] tiles of v^T
        for c in range(NCH):
            pt = psT.tile([I, 128], F32)
            nc.tensor.transpose(pt[:], vt3[:, c, :], ident[:])
            wc = wp.tile([I, 128], F32, tag=f"wT{c}")
            nc.scalar.copy(wc[:], pt[:])
            wT.append(wc)

        # ---- cur = (v^T . cur-delta), scaled by g/||v|| on the PSUM->SBUF
        # copy. One z tile per t-block of TBLK steps, layout [p, (c b tl)],
        # so the vol loop starts after the first t-block's matmuls and the
        # rest of the matmul phase hides under the loop. Matmul windows
        # enumerate (tl, b) columns via a strided rhs AP on delta. ----
        dly = delta[:].rearrange("p (b t) -> p t b", b=BL)  # [126, T, BL]
        ztiles = []
        for tb in range(NTB):
            zt = big.tile([128, NCH * BL * TBLK], F32, tag=f"z{tb}")
            ztiles.append(zt)
            # memory layout (c, b, tl); dims permuted to enumerate (tl, b)
            zv = zt[:].rearrange("p (c b tl) -> p c tl b", c=NCH, b=BL)
            # first t-block in half-windows: the loop's first steps gate on
            # 8 half-size matmuls instead of 8 full ones (subtile deps let
            # A(t<32) start once the first halves are copied).
            halves = ((0, TBLK // 2), (TBLK // 2, TBLK)) if tb == 0 else (
                (0, TBLK),
            )
            for wlo, whi in halves:
                ww = whi - wlo
                for c in range(NCH):
                    ps = psZ.tile([128, BL * TBLK], F32, tag="ps")
                    mm_lhs = wT[c][:]
                    mm_rhs = dly[:, tb * TBLK + wlo : tb * TBLK + whi, :]
                    if MM_F32R:
                        mm_lhs = mm_lhs.bitcast(mybir.dt.float32r)
                        mm_rhs = mm_rhs.bitcast(mybir.dt.float32r)
                    nc.tensor.matmul(
                        ps[:, : ww * BL], lhsT=mm_lhs, rhs=mm_rhs,
                        start=True, stop=True,
                    )
                    # psum cols are (tl, b); write to z at (b*TBLK + tl)
                    nc.scalar.activation(
                        zv[:, c, wlo:whi, :],
                        ps[:, : ww * BL].rearrange(
                            "p (tl b) -> p tl b", b=BL
                        ),
                        Act.Copy,
                        scale=scale[:, c : c + 1],
                    )

        # ---- vol loop: vol_pre overwrites the cur column of z in place.
        # DVE-only; no cross-engine sync inside the loop. ----
        volS = None
        vdec = 1.0 - VOLTAGE_DECAY

        out_r = out.rearrange("b (c p) t -> p c b t", c=NCH)
        for t in range(T):
            tb, tl = divmod(t, TBLK)
            zc = ztiles[tb][:].rearrange(
                "p (c b tl) -> p c b tl", c=NCH, b=BL
            )
            # vol_pre = vdec * vol + cur_t   (written over cur_t).
            # t=0: vol_pre = cur_0 is already in place — skip the op.
            if t > 0:
                nc.vector.scalar_tensor_tensor(
                    zc[:, :, :, tl],
                    volS[:].rearrange("p (c b) -> p c b", c=NCH),
                    vdec,
                    zc[:, :, :, tl],
                    Alu.mult,
                    Alu.add,
                )
            # vol = (vol_pre < 1) * vol_pre   (hard reset); the state after
            # the last step is never consumed — skip it.
            if t < T - 1:
                volS = pvolS.tile([128, NCH * BL], F32, tag="volS")
                nc.vector.scalar_tensor_tensor(
                    volS[:].rearrange("p (c b) -> p c b", c=NCH),
                    zc[:, :, :, tl],
                    1.0,
                    zc[:, :, :, tl],
                    Alu.is_lt,
                    Alu.mult,
                )
            # spikes on ACT (off the DVE path): Sign -> in-place Relu ->
            # DMA out, staging (c, b, tl). Block tb-1 is extracted at the
            # START of block tb (its data is complete and the ACT ops are
            # immediately ready); the final block flushes in quarters as
            # its columns finish so only the last quarter sits on the tail.
            flush = []
            if tb >= 1 and tl == 0:
                flush = [(tb - 1, 0, TBLK)]
            if tb == NTB - 1 and (tl + 1) % (TBLK // 4) == 0:
                q = (tl + 1) // (TBLK // 4) - 1
                flush += [(tb, q * (TBLK // 4), (q + 1) * (TBLK // 4))]
            for ftb, lo, hi in flush:
                w = hi - lo
                zcf = ztiles[ftb][:].rearrange(
                    "p (c b tl) -> p c b tl", c=NCH, b=BL
                )
                ostage = pstage.tile([128, 64 * w], U8, tag=f"os{w}")
                o3 = ostage[:].rearrange(
                    "p (c b tl) -> p c b tl", c=NCH, b=BL
                )
                # spikes are exact 0/1: Sign then Relu (which also narrows
                # to uint8 -> 4x fewer DMA bytes; host widens). A DVE
                # is_ge->uint8 shortcut for the final quarter matched in
                # CoreSim but was WRONG on hardware — keep ACT.
                sstage = pstage.tile([128, 64 * w], F32, tag=f"ss{w}")
                s3 = sstage[:].rearrange(
                    "p (c b tl) -> p c b tl", c=NCH, b=BL
                )
                nc.scalar.activation(
                    s3, zcf[:, :, :, lo:hi], Act.Sign, bias=neg1[:]
                )
                nc.scalar.activation(o3, s3, Act.Relu)
                for c in range(NCH):
                    nc.sync.dma_start(
                        out_r[:, c, :, ftb * TBLK + lo : ftb * TBLK + hi],
                        o3[:, c, :, :],
                    )


_CACHE = {}


def _build():
    if "nc" in _CACHE:
        return _CACHE["nc"]
    nc = bacc.Bacc(
        "TRN2", target_bir_lowering=False, debug=False, num_devices=N_CORES
    )
    x = nc.dram_tensor("x", [BL, I, T], F32, kind="ExternalInput").ap()
    v = nc.dram_tensor("v", [O, I], F32, kind="ExternalInput").ap()
    g = nc.dram_tensor("g", [O], F32, kind="ExternalInput").ap()
    out = nc.dram_tensor("out", [BL, O, T], U8, kind="ExternalOutput").ap()
    with tile.TileContext(nc) as tc:
        _body(tc, x, v, g, out)
    nc.compile()
    _CACHE["nc"] = nc
    return nc


def make_in_maps(x, v_weight, g):
    xr = np.ascontiguousarray(x.reshape(B, I, T))
    v_weight = np.ascontiguousarray(v_weight)
    g = np.ascontiguousarray(g)
    return [
        {
            "x": np.ascontiguousarray(xr[c * BL : (c + 1) * BL]),
            "v": v_weight,
            "g": g,
        }
        for c in range(N_CORES)
    ]


def kernel(x, v_weight, g):
    nc = _build()
    in_maps = make_in_maps(
        np.asarray(x, dtype=np.float32),
        np.asarray(v_weight, dtype=np.float32),
        np.asarray(g, dtype=np.float32),
    )
    last_err = None
    for _attempt in range(3):  # retry: a prior tenant can leave a core wedged
        try:
            res = run_bass_kernel_spmd(nc, in_maps, list(range(N_CORES))).results
            return np.concatenate(
                [res[c]["out"] for c in range(N_CORES)], axis=0
            ).astype(np.float32)
        except Exception as e:  # noqa: BLE001
            last_err = e
    raise last_err
